# revision 28
# baseline (speedup 1.0000x reference)
"""MoE layer (top-2 of 8 experts) on 8 Trainium2 NeuronCores.

Hidden-split expert-pair sharding (B=4,T=1024,D=1024,E=8,K=2,H=4096):
  - Host: gate logits + top-2 + softmax (f64 for a stable argmax), dispatch
    token lists per expert, combine weighted expert outputs (+b2).
  - The 8 experts are sorted by load and paired (1st+2nd heaviest, ...) into
    4 "slots".  Each device core runs 4 pieces: piece p = (one expert of slot
    p's pair, one QUARTER of the hidden dim H).  Cores 0-3 take the heavier
    expert of each pair (quarters 0-3), cores 4-7 the lighter one.  GELU is
    elementwise over H so GEMM1 stays exact; GEMM2 contracts only H/4, so a
    core emits a PARTIAL y that the host combine sums across the 4 quarter
    cores (the combine is doing weighted adds anyway).
  - Per-core PE work = sum of the 4 slot capacities (max load of each pair)
    at 128 cycles/column -- near-perfectly balanced regardless of routing
    skew, with the same per-core weight traffic as plain expert parallelism
    (each core still holds exactly one expert's worth of w1+w2 bytes).
  - bf16 matmul inputs, f32 PSUM accumulation; partial y returned as bf16
    (partials are summed in f32 on host, so the extra quantization is ~0.4%).
  - DMA: x + w1 on the sync-engine queue, w2 + y-out on the scalar-engine
    queue, both issued in exact consumption order so the in-order rings
    stay ahead of the PE.  Warm-up matmuls hold the PE busy through the
    ~8us DMA ring-init so HAM releases the clock gate (2.4 GHz) before the
    real stream starts.
"""

import os

import numpy as np
import ml_dtypes

B, T, D = 4, 1024, 1024
E, K, H = 8, 2, 4 * 1024
N = B * T
P = 128
KD = D // P           # 8 k-tiles in GEMM1 / output d-tiles in GEMM2
NP_ = 4               # pieces (hidden-quarter slots) per core
MQ = H // NP_ // P    # 8 m-tiles per quarter in GEMM1 / k-tiles in GEMM2
HQ = H // NP_         # 1024
BF16 = ml_dtypes.bfloat16

LAST_EXEC_TIME_NS = None
_cached_nc = {}


def _chunks(c, first_small=False):
    """Split a slot's column span into PSUM-bank (<=512 f32) chunks >=250.

    first_small makes the leading chunks 256 cols so the head x DMAs are
    small and the first matmul groups can start as soon as possible.
    """
    out = []
    if first_small:
        while len(out) < 2 and c - 256 >= 250:
            out.append(256)
            c -= 256
    n = -(-c // 512)
    base = c // n
    rem = c - base * n
    out += [base + (1 if i < rem else 0) for i in range(n)]
    res, off = [], 0
    for s in out:
        res.append((off, s))
        off += s
    return res


def _ensure_ntff_hook():
    """Register the axon NTFF profile hook if the image lacks antenv.axon_hooks."""
    import sys
    import types
    try:
        from antenv.axon_hooks import get_axon_ntff_profile_hook
        return get_axon_ntff_profile_hook() is not None
    except ImportError:
        pass
    try:
        import antenv
        from trn_agent_boot.trn_boot import _ntff_profile_via_ctypes
        mod = types.ModuleType("antenv.axon_hooks")
        holder = [None]
        mod.set_axon_ntff_profile_hook = lambda h: holder.__setitem__(0, h)
        mod.get_axon_ntff_profile_hook = lambda: holder[0]
        sys.modules["antenv.axon_hooks"] = mod
        antenv.axon_hooks = mod
        mod.set_axon_ntff_profile_hook(
            _ntff_profile_via_ctypes("/opt/axon/libaxon_pjrt.so"))
        return True
    except Exception:
        return False


def _build(slot_sizes):
    import concourse.mybir as mybir
    import concourse.tile as tile
    from concourse import bacc

    nc = bacc.Bacc(None, target_bir_lowering=False)

    S = list(slot_sizes)
    smax = max(S)
    chunks = [_chunks(S[p], first_small=(p == 0)) for p in range(NP_)]

    xs = {}
    for p in range(NP_):
        for ci, (off, ch) in enumerate(chunks[p]):
            xs[(p, ci)] = nc.declare_dram_parameter(
                f"x{p}_{ci}", [P, KD, ch], mybir.dt.bfloat16, isOutput=False)
    w1 = [nc.declare_dram_parameter(f"w1_{p}", [MQ, P, KD, P],
                                    mybir.dt.bfloat16, isOutput=False)
          for p in range(NP_)]
    w2 = [nc.declare_dram_parameter(f"w2_{p}", [KD, P, MQ, P],
                                    mybir.dt.bfloat16, isOutput=False)
          for p in range(NP_)]
    b1 = nc.declare_dram_parameter("b1", [P, NP_, MQ], mybir.dt.float32,
                                   isOutput=False)
    outs = [nc.declare_dram_parameter(f"out{p}", [P, KD, S[p]],
                                      mybir.dt.bfloat16, isOutput=True)
            for p in range(NP_)]

    GELU = mybir.ActivationFunctionType.Gelu

    with tile.TileContext(nc) as tc, \
         tc.tile_pool(name="singles", bufs=1) as singles, \
         tc.tile_pool(name="w1pool", bufs=8) as w1pool, \
         tc.tile_pool(name="w2pool", bufs=8) as w2pool, \
         tc.tile_pool(name="ypool", bufs=3) as ypool, \
         tc.tile_pool(name="psum", bufs=4, space="PSUM") as psum_pool:

        # PE warm-up: dependency-free matmuls keep the TensorEngine busy
        # through the DMA ring-init window so the HAM clock gate is released
        # (2.4 GHz) roughly when the first x chunk lands (~10.5us).  The
        # memset runs on GpSimd, whose preamble finishes ~1us before Vector's.
        warm_sb = singles.tile([P, 4 * P], mybir.dt.bfloat16)
        nc.gpsimd.memset(warm_sb[:], 0.0)
        ps_warm = psum_pool.tile([P, 4 * P], mybir.dt.float32, name="ps_warm",
                                 tag="ps1")
        for _ in range(8):
            nc.tensor.matmul(ps_warm[:], warm_sb[:, :P], warm_sb[:],
                             start=True, stop=True)

        # --- head DMAs, all on the sync ring, in staircase need order -----
        # The ring drains in issue order at ~310 GB/s; interleaving x chunks
        # between the early w1 tiles makes the delivery curve track the
        # staircase consumption curve.  NOTE: dma_start instructions block
        # the ISSUING engine in order (ring backpressure), so none of these
        # may go on the scalar engine -- they would starve the activations.
        w1_tiles = {}
        x_sbs = {}

        def _w1_dma(p, mo):
            w1_tiles[(p, mo)] = w1pool.tile([P, KD, P], mybir.dt.bfloat16,
                                            name="w1_t")
            nc.sync.dma_start(out=w1_tiles[(p, mo)][:], in_=w1[p][mo])

        def _x_dma(p, ci):
            ch = chunks[p][ci][1]
            x_sbs[(p, ci)] = singles.tile([P, KD, ch], mybir.dt.bfloat16,
                                          name=f"x_{p}_{ci}")
            nc.sync.dma_start(out=x_sbs[(p, ci)][:], in_=xs[(p, ci)][:])

        _w1_dma(0, 0)
        # First x chunk in k-halves, one half per ring: the scalar ring
        # transfers half in parallel with w1 tile 0 + the other half on the
        # sync ring, so neither ring's startup latency gates the first
        # matmul group.  Only ONE DMA may use the scalar engine -- it is
        # issued well before the first ACTIVATE; anything more backs the
        # scalar engine up against the activations (measured: 14us stall).
        x_sbs[(0, 0)] = singles.tile([P, KD, chunks[0][0][1]],
                                     mybir.dt.bfloat16, name="x_0_0")
        nc.scalar.dma_start(out=x_sbs[(0, 0)][:, 0:KD // 2, :],
                            in_=xs[(0, 0)][:, 0:KD // 2, :])
        nc.sync.dma_start(out=x_sbs[(0, 0)][:, KD // 2:KD, :],
                          in_=xs[(0, 0)][:, KD // 2:KD, :])
        _w1_dma(0, 1)
        b1_sb = singles.tile([P, NP_, MQ], mybir.dt.float32)
        nc.sync.dma_start(out=b1_sb[:], in_=b1[:])
        if len(chunks[0]) > 1:
            _x_dma(0, 1)
        _w1_dma(0, 2)
        if len(chunks[0]) > 2:
            _x_dma(0, 2)
        for mo in range(3, MQ):
            _w1_dma(0, mo)
        w2_tiles = {}
        for do in range(KD):
            w2_tiles[(0, do)] = w2pool.tile([P, MQ, P], mybir.dt.bfloat16,
                                            name="w2_t")
            nc.sync.dma_start(out=w2_tiles[(0, do)][:], in_=w2[0][do])
        for p in range(NP_):
            for ci in range(len(chunks[p])):
                if (p, ci) not in x_sbs:
                    _x_dma(p, ci)

        # Double-buffered hidden activations (piece p uses hTs[p % 2]).
        hTs = [singles.tile([P, MQ, smax], mybir.dt.bfloat16, name=f"hT{i}")
               for i in range(2)]

        for p in range(NP_):
            hT = hTs[p % 2]
            # GEMM1 piece p (chunk-outer / m-inner so only one x chunk and
            # the piece's 8 w1 tiles gate the start):
            #   hT[mo*128+r, c] = gelu(sum_k w1q[k,:].T @ xT[k,:] + b1q)
            if p > 0:
                # w1 then w2 for this piece: the dma_start instructions gate
                # on pool-buffer reuse, so the sync engine naturally issues
                # them exactly one piece ahead of consumption.
                for mo in range(MQ):
                    w1_tiles[(p, mo)] = w1pool.tile([P, KD, P],
                                                    mybir.dt.bfloat16,
                                                    name="w1_t")
                    nc.sync.dma_start(out=w1_tiles[(p, mo)][:],
                                      in_=w1[p][mo])
                for do in range(KD):
                    w2_tiles[(p, do)] = w2pool.tile([P, MQ, P],
                                                    mybir.dt.bfloat16,
                                                    name="w2_t")
                    nc.sync.dma_start(out=w2_tiles[(p, do)][:],
                                      in_=w2[p][do])
            n_ch = len(chunks[p])
            if p == 0 and n_ch > 1:
                # Staircase group order: grow the (mo, chunk) rectangle one
                # row/column at a time so the prefix of DMA bytes needed
                # stays under the ring's delivery curve during the head.
                seq = [(0, 0), (1, 0)]
                for ci in range(1, n_ch):
                    seq += [(mo, ci) for mo in range(min(ci + 1, MQ))]
                    if ci + 1 < MQ:
                        seq += [(ci + 1, cj) for cj in range(ci + 1)]
                seq += [(mo, ci) for mo in range(min(n_ch + 1, MQ), MQ)
                        for ci in range(n_ch)]
            else:
                seq = [(mo, ci) for ci in range(n_ch) for mo in range(MQ)]
            for mo, ci in seq:
                off, ch = chunks[p][ci]
                ps1 = psum_pool.tile([P, ch], mybir.dt.float32,
                                     name="ps1")
                for k in range(KD):
                    nc.tensor.matmul(ps1[:], w1_tiles[(p, mo)][:, k, :],
                                     x_sbs[(p, ci)][:, k, :],
                                     start=(k == 0), stop=(k == KD - 1))
                nc.scalar.activation(hT[:, mo, off:off + ch], ps1[:],
                                     GELU, bias=b1_sb[:, p, mo:mo + 1])
            # GEMM2 piece p (partial over this hidden quarter; bias on host):
            #   yT[do*128+r, c] = sum_k w2q[k,:].T @ hT[k,:]
            for do in range(KD):
                w2_t = w2_tiles[(p, do)]
                y_do = ypool.tile([P, smax], mybir.dt.bfloat16, name="y_do")
                last_dma = (p == NP_ - 1) and (do == KD - 1)
                for ci, (off, ch) in enumerate(chunks[p]):
                    ps2 = psum_pool.tile([P, ch], mybir.dt.float32,
                                         name="ps2")
                    for k in range(MQ):
                        nc.tensor.matmul(ps2[:], w2_t[:, k, :],
                                         hT[:, k, off:off + ch],
                                         start=(k == 0), stop=(k == MQ - 1))
                    nc.vector.tensor_copy(y_do[:, off:off + ch], ps2[:])
                    if last_dma:
                        # final (p, do): per-chunk out DMAs on the (by now
                        # idle) SYNC engine shorten the tail -- the blocked
                        # dma_start fires the instant its copy completes,
                        # skipping the scalar engine's issue latency.
                        nc.sync.dma_start(
                            out=outs[p][:, do, off:off + ch],
                            in_=y_do[:, off:off + ch])
                if not last_dma:
                    nc.scalar.dma_start(out=outs[p][:, do, :],
                                        in_=y_do[:, :S[p]])

    nc.compile()
    return nc


def kernel(x, gate_w, gate_b, w1, b1, w2, b2):
    global LAST_EXEC_TIME_NS
    from concourse.bass_utils import run_bass_kernel_spmd

    x = np.asarray(x)
    xf = np.ascontiguousarray(x.reshape(N, D), dtype=np.float32)

    # --- Gate (host, float64 for a stable top-2 selection) ---
    logits = xf.astype(np.float64) @ np.asarray(gate_w).astype(np.float64)
    logits += np.asarray(gate_b).astype(np.float64)
    rows = np.arange(N)
    i1 = np.argmax(logits, axis=1)
    tmp = logits.copy()
    tmp[rows, i1] = -np.inf
    i2 = np.argmax(tmp, axis=1)
    l1 = logits[rows, i1]
    l2 = tmp[rows, i2]
    e2 = np.exp(l2 - l1)          # l1 >= l2
    wa = (1.0 / (1.0 + e2)).astype(np.float32)
    wb = (e2 / (1.0 + e2)).astype(np.float32)

    # --- Dispatch (host): per-expert token lists ---
    sels, wgts = [], []
    for e in range(E):
        sel = np.where((i1 == e) | (i2 == e))[0]
        wgt = np.where(i1[sel] == e, wa[sel], wb[sel])
        sels.append(sel)
        wgts.append(wgt)
    loads = np.array([len(s) for s in sels])

    # Pair experts by sorted load: slot p = (order[2p], order[2p+1]);
    # slot capacity = heavier of the pair; process smallest slot first so the
    # head x DMA is minimal.
    order = np.argsort(-loads, kind="stable")
    pairs = [(int(order[2 * p]), int(order[2 * p + 1])) for p in range(NP_)]
    pairs.sort(key=lambda ab: max(loads[ab[0]], loads[ab[1]]))
    S = tuple(max(256, int(max(loads[a], loads[b]))) for a, b in pairs)
    chunks = [_chunks(S[p], first_small=(p == 0)) for p in range(NP_)]

    # --- Per-core input maps ---
    w1a = np.asarray(w1, dtype=np.float32)
    b1a = np.asarray(b1, dtype=np.float32)
    w2a = np.asarray(w2, dtype=np.float32)
    b2a = np.asarray(b2, dtype=np.float32)

    # Per (expert): padded transposed token block xT [P, KD, S_slot]
    slot_of_expert = {}
    for p, (a, b) in enumerate(pairs):
        slot_of_expert[a] = p
        slot_of_expert[b] = p
    xTs = {}
    for e in range(E):
        p = slot_of_expert[e]
        xe = np.zeros((S[p], D), dtype=np.float32)
        xe[:loads[e]] = xf[sels[e]]
        xTs[e] = np.ascontiguousarray(
            xe.T.reshape(KD, P, S[p]).transpose(1, 0, 2)).astype(BF16)

    in_maps = []
    core_exp = []
    for c in range(E):
        m = {}
        b1_r = np.zeros((P, NP_, MQ), dtype=np.float32)
        exps = []
        for p, (a, b) in enumerate(pairs):
            e = a if c < 4 else b
            q = c % 4
            exps.append((e, q))
            for ci, (off, ch) in enumerate(chunks[p]):
                m[f"x{p}_{ci}"] = np.ascontiguousarray(
                    xTs[e][:, :, off:off + ch])
            w1q = w1a[e][:, q * HQ:(q + 1) * HQ]
            m[f"w1_{p}"] = np.ascontiguousarray(
                w1q.reshape(KD, P, MQ, P).transpose(2, 1, 0, 3)).astype(BF16)
            w2q = w2a[e][q * HQ:(q + 1) * HQ, :]
            m[f"w2_{p}"] = np.ascontiguousarray(
                w2q.reshape(MQ, P, KD, P).transpose(2, 1, 0, 3)).astype(BF16)
            b1_r[:, p, :] = b1a[e][q * HQ:(q + 1) * HQ].reshape(MQ, P).T
        m["b1"] = b1_r
        core_exp.append(exps)
        in_maps.append(m)

    if S not in _cached_nc:
        _cached_nc[S] = _build(S)
    nc = _cached_nc[S]

    trace = os.environ.get("MOE_KERNEL_PROFILE", "0") == "1"
    if trace:
        trace = _ensure_ntff_hook()
    res = None
    for attempt in range(3):
        try:
            res = run_bass_kernel_spmd(nc, in_maps, core_ids=list(range(E)),
                                       trace=trace and attempt == 0)
            break
        except Exception:
            # Device-unrecoverable NRT errors are transient here; retry with
            # a fresh PJRT client (last attempt re-raises).
            if attempt == 2:
                raise
            try:
                import jax
                jax.clear_caches()
                jax._src.api.clear_backends()
            except Exception:
                pass
    LAST_EXEC_TIME_NS = res.exec_time_ns

    # --- Combine (host): sum the 4 hidden-quarter partials per expert,
    # add b2, then weighted scatter-add into the output. ---
    out_acc = np.zeros((N, D), dtype=np.float32)
    for e in range(E):
        p = slot_of_expert[e]
        cores = range(0, 4) if e == pairs[p][0] else range(4, 8)
        ysum = np.zeros((D, loads[e]), dtype=np.float32)
        for c in cores:
            yT = np.asarray(res.results[c][f"out{p}"]).astype(np.float32)
            ysum += yT.transpose(1, 0, 2).reshape(D, S[p])[:, :loads[e]]
        y = ysum.T + b2a[e]
        out_acc[sels[e]] += wgts[e][:, None] * y

    return out_acc.reshape(B, T, D)
